# revision 2
# baseline (speedup 1.0000x reference)
"""Trainium2 Bass kernel for the correlation-softargmax flow module, v3.

Math (per batch b, query pixel q=(y,x)):
  c1 = l2norm_C(feature1), warp = l2norm_C(feature2)
  s[l,q] = <3x3 patch of warp at l, 3x3 patch of c1 at q>    (D = 32*9 = 288)
  p = softmax_l(10*s);  flow = (E_p[ix_l] - x, E_p[iy_l] - y)

Only Z = sum_l exp, Sy = sum_l exp*iy, Sx = sum_l exp*ix are needed per q
(flash-attention style; exp(10*s - 30) needs no running max since |10*s| <= 90).

Sharding: 8 cores = 4 batches x 2 query-row halves.

Structure (v2+v3):
- The PE never waits on the scalar engine: stats matmuls lag 4 tiles behind
  the score matmuls, exp is batched over 2-tile [128,1024] PSUM pairs, 6 PSUM
  banks rotate for scores. (v1 stalled the PE every tile, which kept the HAM
  clock throttle oscillating and every matmul at half rate.)
- Patch tensors stack the 3 dx taps on partitions (96 = 3x32); dy taps are
  row offsets in the matmul APs (rows are 64 wide and contiguous so the
  stationary window coalesces to one free dim, a BIR requirement).
- v3 startup: image1 is normalized first so its patch copies overlap image2's
  normalization; sum-of-squares lands on 3x32 partition groups so one
  Ln/Exp activation pass covers up to 6 pixel chunks; sk copies are split
  into row blocks so the main loop starts after the first block.
- v3 epilogue: stats are transposed to [128, 4] via two small DMAs so the
  reciprocal runs 128 lanes wide (the [1,512] form cost 3.3us on one lane).
"""

import sys

import numpy as np

sys.path.insert(0, "/opt/trn_rl_repo")

import concourse.bass as bass  # noqa: E402
import concourse.mybir as mybir  # noqa: E402
import concourse.tile as tile  # noqa: E402
from concourse import bacc, bass_utils  # noqa: E402

F32 = mybir.dt.float32
F32R = mybir.dt.float32r
F16 = mybir.dt.float16
BF16 = mybir.dt.bfloat16

B, C, H, W = 4, 32, 64, 64
L = H * W              # 4096 match locations
NQ = L // 2            # queries per core
QROWS = H // 2         # query rows per core
N_CORES = 8
SCALE = 10.0
SHIFT = -30.0          # exp(10*s - 30): |10*s|<=90 so no overflow, and a row's
                       # max 10*s is never < -60 so Z stays far above underflow
EPS = 1e-12

N_LT = L // 128        # 32 l-tiles of 128 locations
N_QT = NQ // 512       # 4 q-tiles of 512 queries
LAG = 4                # stats matmul runs LAG tiles behind the score matmuls

_NC_CACHE = {}
_LAST_RES = None


def _build_nc():
    nc = bacc.Bacc(None, target_bir_lowering=False)

    f1h = nc.dram_tensor("f1h", [C, QROWS + 2, W], F32, kind="ExternalInput")
    f2 = nc.dram_tensor("f2", [C, H, W], F32, kind="ExternalInput")
    w3 = nc.dram_tensor("w3", [128, 96], F32, kind="ExternalInput")
    yqt = nc.dram_tensor("yqt", [128, 16], F32, kind="ExternalInput")
    xqt = nc.dram_tensor("xqt", [128, 16], F32, kind="ExternalInput")
    outp = nc.dram_tensor("outp", [2, NQ], F32, kind="ExternalOutput")

    n1 = (QROWS + 2) * W   # 2176 pixels in the f1 halo slab

    with tile.TileContext(nc) as tc:
        with tc.tile_pool(name="big", bufs=1) as big, \
             tc.tile_pool(name="work", bufs=2) as work, \
             tc.tile_pool(name="small", bufs=1) as small, \
             tc.tile_pool(name="pp", bufs=3) as pp, \
             tc.tile_pool(name="epi", bufs=2) as epi, \
             tc.tile_pool(name="prs", bufs=3, space="PSUM") as prs, \
             tc.tile_pool(name="stps", bufs=2, space="PSUM") as stps:

            # ---- load inputs (image1 first: its normalize runs first) ----
            raw1 = big.tile([C, n1], F32, tag="raw1")
            nc.sync.dma_start(out=raw1, in_=f1h[:, :, :].rearrange("c h w -> c (h w)"))
            raw2 = big.tile([C, L], F32, tag="raw2")
            nc.scalar.dma_start(out=raw2, in_=f2[:, :, :].rearrange("c h w -> c (h w)"))
            w3f = small.tile([128, 96], F32, tag="w3f")
            nc.gpsimd.dma_start(out=w3f, in_=w3[:, :])
            w3r = small.tile([128, 96], BF16, tag="w3r")
            nc.vector.tensor_copy(w3r, w3f)
            xqs = small.tile([128, 16], F32, tag="xqs")
            nc.gpsimd.dma_start(out=xqs, in_=xqt[:, :])
            yqs = small.tile([128, 16], F32, tag="yqs")
            nc.gpsimd.dma_start(out=yqs, in_=yqt[:, :])

            # PE warm-up burst: ~12 back-to-back matmuls during the input
            # loads re-arm the HAM clock gate to 8/8 before real work starts
            warm = small.tile([128, 512], BF16, tag="warm")
            nc.vector.memset(warm, 0.125)
            wpr = None
            for wi in range(12):
                if wi % 2 == 0:
                    wpr = prs.tile([128, 1024], F32, tag="pair",
                                   name=f"wpr{(wi // 2) % 3}")
                nc.tensor.matmul(wpr[:, 512 * (wi % 2):512 * (wi % 2) + 512],
                                 warm[:, 0:128], warm, start=True, stop=True)

            onesf = small.tile([C, 128], F32, tag="onesf")
            nc.vector.memset(onesf, 1.0)
            ones32 = small.tile([C, 128], F32R, tag="ones32")
            nc.vector.tensor_copy(ones32, onesf)
            shiftc = small.tile([128, 1], F32, tag="shiftc")
            nc.vector.memset(shiftc, SHIFT)
            eps2c = small.tile([128, 1], F32, tag="eps2c")
            nc.vector.memset(eps2c, EPS * EPS)

            # ---- padded normalized images (fp16), x-padding zeroed. pad2 is
            # split into two row-halves so each half's patch copies can start
            # as soon as that half's normalize finishes ----
            HB = 33
            pad2a = big.tile([C, HB, W + 2], F16, tag="pad2a")
            pad2b = big.tile([C, HB, W + 2], F16, tag="pad2b")
            pad1 = big.tile([C, QROWS + 2, W + 2], F16, tag="pad1")
            nc.vector.memset(pad2a[:, 0:1, :], 0.0)
            nc.vector.memset(pad2a[:, 1:HB, 0:1], 0.0)
            nc.vector.memset(pad2a[:, 1:HB, W + 1:W + 2], 0.0)
            nc.vector.memset(pad2b[:, HB - 1:HB, :], 0.0)
            nc.vector.memset(pad2b[:, 0:HB - 1, 0:1], 0.0)
            nc.vector.memset(pad2b[:, 0:HB - 1, W + 1:W + 2], 0.0)
            nc.gpsimd.memset(pad1[:, :, 0:1], 0.0)
            nc.gpsimd.memset(pad1[:, :, W + 1:W + 2], 0.0)

            # ---- l2 normalize over C: ss via replicated ones-matmul (the HW
            # verifier requires equal SBUF base partitions across tensor-op
            # inputs, so the scale row must land at base 0); rsqrt =
            # exp(-0.5*ln(ss+eps^2)) since Rsqrt itself is blocked in bass ----
            eng_cnt = [0]

            def normalize(raw, npix, pad, row0, img):
                offs = [(off, min(512, npix - off))
                        for off in range(0, npix, 512)]
                raw3 = raw.rearrange("c (h w) -> c h w", w=W)
                for k0 in range(0, len(offs), 2):
                    grp = offs[k0:k0 + 2]
                    pr = prs.tile([128, 1024], F32, tag="pair",
                                  name=f"npair{img}{k0}")
                    tot = 0
                    for k, (off, n) in enumerate(grp):
                        sqc = work.tile([C, 512], F32R, tag="sqc", name="sqc")
                        eng = nc.vector if eng_cnt[0] % 2 == 0 else nc.gpsimd
                        eng_cnt[0] += 1
                        eng.tensor_mul(sqc[:, :n], raw[:, off:off + n],
                                       raw[:, off:off + n])
                        nc.tensor.matmul(pr[:, 512 * k:512 * k + n], ones32,
                                         sqc[:, :n], start=True, stop=True)
                        tot = 512 * k + n
                    lnr = work.tile([128, 1024], F32, tag="lnr", name="lnr")
                    nc.scalar.activation(lnr[:, :tot], pr[:, :tot],
                                         mybir.ActivationFunctionType.Ln,
                                         bias=eps2c)
                    rb = work.tile([128, 1024], F32, tag="rb", name="rb")
                    nc.scalar.activation(rb[:, :tot], lnr[:, :tot],
                                         mybir.ActivationFunctionType.Exp,
                                         scale=-0.5)
                    for k, (off, n) in enumerate(grp):
                        rows = n // W
                        r0 = off // W
                        # muls gate the patch copies: keep them on the fast
                        # vector engine (gpsimd runs f32 muls ~2x slower)
                        eng = nc.vector
                        eng.tensor_mul(
                            pad[:, row0 + r0:row0 + r0 + rows, 1:W + 1],
                            raw3[:, r0:r0 + rows, :],
                            rb[0:C, 512 * k:512 * k + n].rearrange(
                                "c (h w) -> c h w", w=W))

            # ---- dx-stacked patch tensors: S[32g+c, Y, x] = pad[c, Y, x+g];
            # dy taps become row offsets in the matmul APs ----
            normalize(raw1, n1, pad1, row0=0, img=1)

            sq = big.tile([3 * C, QROWS + 2, W], F16, tag="sq")
            nc.sync.dma_start(out=sq[0:32, :, :], in_=pad1[:, :, 0:W])
            nc.scalar.dma_start(out=sq[32:64, :, :], in_=pad1[:, :, 1:W + 1])
            nc.gpsimd.dma_start(out=sq[64:96, :, :], in_=pad1[:, :, 2:W + 2])

            # sk split into two tiles so the first 14 l-tiles' matmuls (which
            # only need rows 0-32, all from pad2a) are not blocked by pad2b's
            # normalize: sk0 = rows [0,33), sk1 = rows [28,66) (local -28)
            SK1R = 28
            sk0 = big.tile([3 * C, HB, W], F16, tag="sk0")
            sk1 = big.tile([3 * C, 2 * HB - SK1R, W], F16, tag="sk1")

            normalize(raw2[:, 0:L // 2], L // 2, pad2a, row0=1, img=2)
            for g, eng in enumerate([nc.sync, nc.scalar, nc.gpsimd]):
                eng.dma_start(out=sk0[32 * g:32 * g + 32, :, :],
                              in_=pad2a[:, :, g:g + W])
                eng.dma_start(out=sk1[32 * g:32 * g + 32, 0:HB - SK1R, :],
                              in_=pad2a[:, SK1R:HB, g:g + W])

            normalize(raw2[:, L // 2:L], L // 2, pad2b, row0=0, img=3)
            for g, eng in enumerate([nc.gpsimd, nc.sync, nc.scalar]):
                eng.dma_start(out=sk1[32 * g:32 * g + 32, HB - SK1R:, :],
                              in_=pad2b[:, :, g:g + W])

            # ---- main loop: 128 tiles of [128 l, 512 q] ----
            pairs = {}
            p_sb = {}
            stats_t = {}

            def epilogue(qt, stats):
                # stats rows are [Z, Sy, Sx] on partitions 0-2; transpose to
                # [128, 4, 3] (q = 128k + p) so everything runs 128 lanes wide
                # q = 4p + k keeps every DMA side contiguous
                st3 = epi.tile([3, 512], F32, tag="st3")
                nc.vector.tensor_copy(st3, stats)
                stt = epi.tile([128, 3, 4], F32, tag="stt")
                for s, eng in enumerate([nc.sync, nc.scalar, nc.gpsimd]):
                    eng.dma_start(out=stt[:, s:s + 1, :],
                                  in_=st3[s:s + 1, :])
                rz = epi.tile([128, 1, 4], F32, tag="rz")
                nc.vector.reciprocal(rz, stt[:, 0:1, :])
                fw = epi.tile([128, 1, 4], F32, tag="fw")
                nc.vector.tensor_mul(fw, stt[:, 2:3, :], rz)
                nc.vector.tensor_sub(
                    fw, fw, xqs[:, 4 * qt:4 * qt + 4].rearrange(
                        "p (o k) -> p o k", o=1))
                fh = epi.tile([128, 1, 4], F32, tag="fh")
                nc.vector.tensor_mul(fh, stt[:, 1:2, :], rz)
                nc.vector.tensor_sub(
                    fh, fh, yqs[:, 4 * qt:4 * qt + 4].rearrange(
                        "p (o k) -> p o k", o=1))
                nc.sync.dma_start(
                    out=outp[0:1, 512 * qt:512 * qt + 512], in_=fw)
                nc.sync.dma_start(
                    out=outp[1:2, 512 * qt:512 * qt + 512], in_=fh)

            def stats_mm(j):
                qt_j, lt_j = divmod(j, N_LT)
                if lt_j == 0:
                    stats_t[qt_j] = stps.tile([3, 512], F32, tag="stats",
                                              name=f"stats{qt_j % 2}")
                pr_j, half_j = divmod(j, 2)
                nc.tensor.matmul(
                    stats_t[qt_j], w3r[:, 3 * lt_j:3 * lt_j + 3],
                    p_sb[pr_j][:, 512 * half_j:512 * half_j + 512],
                    start=(lt_j == 0), stop=(lt_j == N_LT - 1))
                if half_j == 1:
                    del p_sb[pr_j]
                if lt_j == N_LT - 1:
                    epilogue(qt_j, stats_t.pop(qt_j))

            total = N_QT * N_LT
            for i in range(total + 4):
                if i < total:
                    qt, lt = divmod(i, N_LT)
                    pr_i, half = divmod(i, 2)
                    if half == 0:
                        pairs[pr_i] = prs.tile([128, 1024], F32, tag="pair",
                                               name=f"pair{pr_i % 3}")
                    s_sl = pairs[pr_i][:, 512 * half:512 * half + 512]
                    skt, ro = (sk0, 0) if lt <= 13 else (sk1, SK1R)
                    r = dy0 = 2 * lt - ro
                    for dy in range(3):
                        nc.tensor.matmul(
                            s_sl,
                            skt[:, dy + r:dy + r + 2, :],
                            sq[:, dy + 8 * qt:dy + 8 * qt + 8, :],
                            start=(dy == 0), stop=(dy == 2))
                    if half == 1:
                        p = pp.tile([128, 1024], BF16, tag="p",
                                    name=f"p{pr_i % 3}")
                        nc.scalar.activation(p, pairs[pr_i],
                                             mybir.ActivationFunctionType.Exp,
                                             bias=shiftc, scale=SCALE)
                        p_sb[pr_i] = p
                        del pairs[pr_i]
                # stats for a whole exp pair together: the second matmul
                # waits on the same semaphore as the first, so only one
                # wait-check lands on the PE stream
                if i % 2 == 1 and i >= 5:
                    stats_mm(i - 5)
                    stats_mm(i - 4)

    nc.finalize()
    return nc


def _host_consts():
    p = np.arange(128)
    w3 = np.zeros((128, 96), np.float32)
    for t in range(32):
        w3[:, 3 * t] = 1.0
        w3[:, 3 * t + 1] = 2 * t + p // 64   # global iy of l = 128*lt + p
        w3[:, 3 * t + 2] = p % 64            # global ix
    q = np.arange(NQ)
    xq = (q % W).astype(np.float32)
    ly = (q // W).astype(np.float32)
    return w3, xq, ly


def _transpose_q(v):
    # [NQ] -> [128, 16]: vT[p, 4*qt+k] = v[512*qt + 4*p + k]
    return np.ascontiguousarray(
        v.reshape(4, 128, 4).transpose(1, 0, 2).reshape(128, 16))


def kernel(feature1, feature2):
    feature1 = np.ascontiguousarray(feature1, np.float32)
    feature2 = np.ascontiguousarray(feature2, np.float32)
    w3, xq, ly = _host_consts()

    f1p = np.zeros((B, C, H + 2, W), np.float32)
    f1p[:, :, 1:H + 1, :] = feature1

    xqT = _transpose_q(xq)
    in_maps = []
    for core in range(N_CORES):
        b, h = divmod(core, 2)
        in_maps.append({
            "f1h": np.ascontiguousarray(f1p[b, :, h * QROWS:h * QROWS + QROWS + 2, :]),
            "f2": np.ascontiguousarray(feature2[b]),
            "w3": w3,
            "yqt": _transpose_q((ly + h * QROWS).astype(np.float32)),
            "xqt": xqT,
        })

    if "nc" not in _NC_CACHE:
        _NC_CACHE["nc"] = _build_nc()
    res = bass_utils.run_bass_kernel_spmd(
        _NC_CACHE["nc"], in_maps, core_ids=list(range(N_CORES)))
    global _LAST_RES
    _LAST_RES = res

    out = np.zeros((B, 2, H, W), np.float32)
    for core in range(N_CORES):
        b, h = divmod(core, 2)
        out[b, :, h * QROWS:(h + 1) * QROWS, :] = (
            res.results[core]["outp"].reshape(2, QROWS, W))
    return out
